# revision 1
# baseline (speedup 1.0000x reference)
"""BERT attention block (quirky variant: energies=Q@V^T, attended=W@K) on 8 trn2
NeuronCores.

Sharding: zero-collective decomposition. Core c handles batch b=c//4 and query
rows [512*(c%4), 512*(c%4+1)). Each core computes Q projection for its slice,
full K/V projections for its batch (duplicated across the 4 cores of a batch --
far cheaper than any cross-core collective on this platform), attention for all
16 heads restricted to its query rows, output projection, residual add and
LayerNorm. Host concatenates the 8 disjoint (512, 1024) output slices.

All matmuls run in float32r (TF32-like, ~1.5e-4 matmul rel err) which streams at
1 cycle/row. Layouts keep every reduction on the free axis:
  - Q^T, V^T in (head_dim, seq) layout; K in (seq, head_dim) layout with a
    ones-column per head so the attended matmul also emits the softmax
    normalizer (row 64 of the 65-row PSUM output).
  - E^T tiles for a head pair land in one 2-bank PSUM tile; a single fused
    ScalarE exp (scale 0.25 = 1/sqrt(16)) evacuates both; the attended matmuls
    accumulate over the 16 k-blocks.
  - softmax division is deferred: unnormalized attended rows are scaled by
    1/sumexp (PE one-hot broadcast) just before the output projection.
  - the second half of the K projection is software-pipelined into the
    attention of head-pairs 0-3 to fill PE idle while ScalarE (exp) paces.
"""

import sys

sys.path.insert(0, "/opt/trn_rl_repo")

import numpy as np

B, L, H = 2, 2048, 1024
NH, HEAD = 16, 64
NC = 8
QS = L // 4  # query rows per core
KT = H // 128  # contraction tiles for the projections
LB = L // 128  # key-position blocks
LN_EPS = 1e-12

_programs = {}


def _build(has_bias, has_mask, has_gamma, has_beta):
    import concourse.mybir as mybir
    import concourse.tile as tile
    from concourse import bacc

    F32 = mybir.dt.float32
    F32R = mybir.dt.float32r
    AF = mybir.ActivationFunctionType
    AX = mybir.AxisListType
    OP = mybir.AluOpType

    nc = bacc.Bacc("TRN2", target_bir_lowering=False, debug=False, num_devices=NC)

    embT_d = nc.dram_tensor("embT", [H, L], F32R, kind="ExternalInput")
    embq_d = nc.dram_tensor("embq", [H, QS], F32R, kind="ExternalInput")
    wq_d = nc.dram_tensor("wq", [H, H], F32R, kind="ExternalInput")
    wk_d = nc.dram_tensor("wk", [H, H], F32R, kind="ExternalInput")
    wv_d = nc.dram_tensor("wv", [H, H], F32R, kind="ExternalInput")
    wo_d = nc.dram_tensor("wo", [H, H], F32R, kind="ExternalInput")
    ones_d = nc.dram_tensor("onescol", [128, 64], F32R, kind="ExternalInput")
    zrow_d = nc.dram_tensor("zrow", [NH, QS], F32R, kind="ExternalInput")
    res_d = nc.dram_tensor("res", [QS, H], F32, kind="ExternalInput")
    if has_mask:
        mask_d = nc.dram_tensor("maskpk", [128, LB], F32, kind="ExternalInput")
    if has_bias:
        bq_d = nc.dram_tensor("bqr", [1, H], F32R, kind="ExternalInput")
        bk_d = nc.dram_tensor("bkr", [1, H], F32R, kind="ExternalInput")
        bv_d = nc.dram_tensor("bvr", [1, H], F32R, kind="ExternalInput")
        onesrow_d = nc.dram_tensor("onesrow", [1, L], F32R, kind="ExternalInput")
    if has_gamma:
        gam_d = nc.dram_tensor("gam", [128, H], F32, kind="ExternalInput")
    if has_beta:
        bet_d = nc.dram_tensor("bet", [128, H], F32, kind="ExternalInput")
    out_d = nc.dram_tensor("out", [QS, H], F32, kind="ExternalOutput")
    oh2_d = nc.dram_tensor("oh2", [NH, 8 * 128], F32R, kind="ExternalInput")
    rcp_d = nc.dram_tensor("rcpD", [NH, QS], F32R)
    # unnormalized attended^T, spilled head-pair-stacked: rows [128p, 128p+128)
    apair_d = nc.dram_tensor("apairD", [8 * 128, QS], F32R)
    # Q^T rows 512-1023 (pairs 4-7), spilled until phase C
    qsp_d = nc.dram_tensor("qspD", [512, QS], F32R)

    with tile.TileContext(nc) as tc:
        with (
            tc.tile_pool(name="persist", bufs=1) as pp,
            tc.tile_pool(name="ps_pp", bufs=2, space="PSUM") as ps_pp,
            tc.tile_pool(name="ps_pe", bufs=2, space="PSUM") as ps_pe,
            tc.tile_pool(name="ps_pa", bufs=2, space="PSUM") as ps_pa,
        ):
            qt = [pp.tile([128, QS], F32R, name=f"qt{t}") for t in range(4)]
            qt = qt + [None] * 4
            ones16 = pp.tile([128, 64], F32R, name="ones16")
            nc.sync.dma_start(ones16[:], ones_d[:])
            if has_mask:
                maskt = pp.tile([128, LB], F32, name="maskt")
                nc.sync.dma_start(maskt[:], mask_d[:])
            if has_bias:
                ones1 = pp.tile([1, L], F32R, name="ones1")
                nc.sync.dma_start(ones1[:], onesrow_d[:])
                bqr = pp.tile([1, H], F32R, name="bqr")
                bkr = pp.tile([1, H], F32R, name="bkr")
                bvr = pp.tile([1, H], F32R, name="bvr")
                nc.sync.dma_start(bqr[:], bq_d[:])
                nc.sync.dma_start(bkr[:], bk_d[:])
                nc.sync.dma_start(bvr[:], bv_d[:])

            def kproj_mm(pk, kh_tiles, wkh, dch, lb, kt):
                """One K-projection matmul (+ evacuation on the last)."""
                nc.tensor.matmul(
                    pk[:],
                    embt[kt][:, 128 * lb : 128 * (lb + 1)],
                    wkh[kt][:],
                    start=(kt == 0),
                    stop=(kt == KT - 1 and not has_bias),
                )
                if kt == KT - 1:
                    if has_bias:
                        nc.tensor.matmul(
                            pk[:],
                            ones1[:, :128],
                            bkr[:, 512 * dch : 512 * (dch + 1)],
                            start=False,
                            stop=True,
                        )
                    dst = kh_tiles[lb][:].rearrange("p (h x) -> p h x", x=65)[
                        :, :, 0:64
                    ]
                    src = pk[:].rearrange("p (h x) -> p h x", x=64)
                    nc.vector.tensor_copy(dst, src)
                    nc.vector.tensor_copy(
                        kh_tiles[lb][:].rearrange("p (h x) -> p h x", x=65)[
                            :, :, 64:65
                        ],
                        ones16[:, 0:8].rearrange("p (h x) -> p h x", x=1),
                    )

            def make_vslot(p, ap, tag, bufs):
                """Allocate the V^T pair tile + weight tiles, start the DMAs."""
                vt = ap.tile([128, L], F32R, tag=tag, bufs=bufs, name=f"vt{p}")
                wvp = [
                    ap.tile(
                        [128, 128], F32R, tag=f"wv{i}", bufs=bufs,
                        name=f"wvp{p}_{i}",
                    )
                    for i in range(KT)
                ]
                for i in range(KT):
                    nc.gpsimd.dma_start(
                        wvp[i][:],
                        wv_d[128 * i : 128 * (i + 1), 128 * p : 128 * (p + 1)],
                    )
                return vt, wvp

            def vproj_mm(vt, wvp, p, nch, kt):
                """One V^T-projection matmul (+ evacuation on the last)."""
                if kt == 0:
                    vslot_box[0] = ps_pp.tile(
                        [128, 512], F32, tag="pp", name=f"pv{p}_{nch}"
                    )
                pv = vslot_box[0]
                nc.tensor.matmul(
                    pv[:],
                    wvp[kt][:],
                    embt[kt][:, 512 * nch : 512 * (nch + 1)],
                    start=(kt == 0),
                    stop=(kt == KT - 1 and not has_bias),
                )
                if kt == KT - 1:
                    if has_bias:
                        nc.tensor.matmul(
                            pv[:],
                            bvr[:, 128 * p : 128 * (p + 1)],
                            ones1[:, :512],
                            start=False,
                            stop=True,
                        )
                    nc.vector.tensor_copy(vt[:, 512 * nch : 512 * (nch + 1)], pv[:])

            vslot_box = [None]

            def emit_vproj(vt, wvp, p):
                for nch in range(4):
                    for kt in range(KT):
                        vproj_mm(vt, wvp, p, nch, kt)

            def emit_attn(p, vt, ap, kh_of, filler, direct=False):
                """Attention for head pair p on an already-projected vt."""
                h0, h1 = 2 * p, 2 * p + 1
                pa0 = ps_pa.tile([65, QS], F32, tag="pa", name=f"pa{h0}")
                pa1 = ps_pa.tile([65, QS], F32, tag="pa", name=f"pa{h1}")
                for kb in range(LB):
                    pe2 = ps_pe.tile([128, 2 * QS], F32, tag="pe", name=f"pe{p}_{kb}")
                    nc.tensor.matmul(
                        pe2[:, 0:QS],
                        vt[0:64, 128 * kb : 128 * (kb + 1)],
                        qt[p][0:64, :],
                        start=True,
                        stop=True,
                    )
                    nc.tensor.matmul(
                        pe2[:, QS : 2 * QS],
                        vt[64:128, 128 * kb : 128 * (kb + 1)],
                        qt[p][64:128, :],
                        start=True,
                        stop=True,
                    )
                    ex = ap.tile(
                        [128, 2 * QS], F32R, tag="ex", bufs=2, name=f"ex{p}_{kb}"
                    )
                    bias = maskt[:, kb : kb + 1] if has_mask else 0.0
                    nc.scalar.activation(ex[:], pe2[:], AF.Exp, bias=bias, scale=0.25)
                    kt0, l0 = kh_of(h0)
                    kt1, l1 = kh_of(h1)
                    nc.tensor.matmul(
                        pa0[:],
                        kt0[kb][:, 65 * l0 : 65 * (l0 + 1)],
                        ex[:, 0:QS],
                        start=(kb == 0),
                        stop=(kb == LB - 1),
                    )
                    nc.tensor.matmul(
                        pa1[:],
                        kt1[kb][:, 65 * l1 : 65 * (l1 + 1)],
                        ex[:, QS : 2 * QS],
                        start=(kb == 0),
                        stop=(kb == LB - 1),
                    )
                    if filler is not None:
                        filler(kb)
                scs = []
                for r, pa in ((0, pa0), (1, pa1)):
                    h = 2 * p + r
                    sc = ap.tile([65, QS], F32R, tag="sc", bufs=2, name=f"sc{h}")
                    nc.vector.tensor_copy(sc[:], pa[:])
                    rcw = ap.tile([65, QS], F32, tag="rcw", bufs=2, name=f"rcw{h}")
                    nc.vector.reciprocal_approx_fast(
                        rcw[:], sc[:].bitcast(F32)
                    )
                    nc.vector.tensor_copy(sc[64:65, :], rcw[64:65, :])
                    nc.sync.dma_start(rcp_d[h : h + 1, :], sc[64:65, :])
                    if direct:
                        nc.sync.dma_start(
                            apr[p][64 * r : 64 * (r + 1), :], sc[0:64, :]
                        )
                    else:
                        nc.sync.dma_start(
                            apair_d[128 * p + 64 * r : 128 * p + 64 * (r + 1), :],
                            sc[0:64, :],
                        )
                    scs.append(sc)
                return scs

            with tc.tile_pool(name="embp", bufs=1) as ep:
                embt = [ep.tile([128, L], F32R, name=f"embt{i}") for i in range(KT)]

                # ---- Q projection: qt[t] = (emb_slice @ Wq)^T rows [128t,128t+128)
                with tc.tile_pool(name="qproj", bufs=1) as qp:
                    embqt = [
                        qp.tile([128, QS], F32R, name=f"embq{i}") for i in range(KT)
                    ]
                    wqt = [qp.tile([128, H], F32R, name=f"wqt{i}") for i in range(KT)]
                    # Q-proj inputs first (embq in parallel on gpsimd), then
                    # the big embT tiles (needed only from K-proj on).
                    for i in range(KT):
                        nc.sync.dma_start(
                            embqt[i][:], embq_d[128 * i : 128 * (i + 1), :]
                        )
                        nc.sync.dma_start(wqt[i][:], wq_d[128 * i : 128 * (i + 1), :])
                    for i in range(KT):
                        nc.sync.dma_start(
                            embt[i][:], embT_d[128 * i : 128 * (i + 1), :]
                        )
                    for m in range(8):
                        pq = ps_pp.tile([128, QS], F32, tag="pp", name=f"pq{m}")
                        for kt in range(KT):
                            nc.tensor.matmul(
                                pq[:],
                                wqt[kt][:, 128 * m : 128 * (m + 1)],
                                embqt[kt][:],
                                start=(kt == 0),
                                stop=(kt == KT - 1 and not has_bias),
                            )
                        if has_bias:
                            nc.tensor.matmul(
                                pq[:],
                                bqr[:, 128 * m : 128 * (m + 1)],
                                ones1[:, :QS],
                                start=False,
                                stop=True,
                            )
                        if m < 4:
                            nc.vector.tensor_copy(qt[m][:], pq[:])
                        else:
                            qsc = qp.tile(
                                [128, QS], F32R, tag="qsc", bufs=4, name=f"qsc{m}"
                            )
                            nc.vector.tensor_copy(qsc[:], pq[:])
                            nc.sync.dma_start(
                                qsp_d[128 * (m - 4) : 128 * (m - 3), :], qsc[:]
                            )

                with tc.tile_pool(name="mid", bufs=1) as mid:
                    # kh1: heads 8-15, written via the interleaved filler
                    kh1 = [
                        mid.tile([128, 8 * 65], F32R, name=f"kh1_{lb}")
                        for lb in range(LB)
                    ]

                    with tc.tile_pool(name="kh0p", bufs=1) as kh0p:
                        kh0 = [
                            kh0p.tile([128, 8 * 65], F32R, name=f"kh0_{lb}")
                            for lb in range(LB)
                        ]
                        wvp0 = [
                            kh0p.tile([128, 128], F32R, name=f"wv0e_{i}")
                            for i in range(KT)
                        ]
                        for i in range(KT):
                            nc.sync.dma_start(
                                wvp0[i][:], wv_d[128 * i : 128 * (i + 1), 0:128]
                            )
                        # ---- K projection, heads 0-7 (upfront)
                        with tc.tile_pool(name="kproj", bufs=1) as kp:
                            wkh0 = [
                                kp.tile([128, 512], F32R, name=f"wkh0_{i}")
                                for i in range(KT)
                            ]
                            for i in range(KT):
                                nc.gpsimd.dma_start(
                                    wkh0[i][:], wk_d[128 * i : 128 * (i + 1), 0:512]
                                )
                            for lb in range(LB):
                                pk = ps_pp.tile(
                                    [128, 512], F32, tag="pp", name=f"pk0_{lb}"
                                )
                                for kt in range(KT):
                                    kproj_mm(pk, kh0, wkh0, 0, lb, kt)

                        # ---- pairs 0-3 with K-proj heads 8-15 interleaved
                        with tc.tile_pool(name="attnB", bufs=1) as apB:
                            wkh1 = [
                                apB.tile([128, 512], F32R, name=f"wkh1_{i}")
                                for i in range(KT)
                            ]
                            for i in range(KT):
                                nc.gpsimd.dma_start(
                                    wkh1[i][:],
                                    wk_d[128 * i : 128 * (i + 1), 512:1024],
                                )
                            pk1_box = [None]

                            def kh_of_B(h):
                                return (kh0, h) if h < 8 else (kh1, h - 8)

                            vt_b = apB.tile([128, L], F32R, tag="vt", name="vt0")
                            emit_vproj(vt_b, wvp0, 0)
                            for p in range(4):
                                if p < 3:
                                    vt_n, wvp_n = make_vslot(p + 1, apB, "vt", 1)

                                def fillerB(kb, p=p):
                                    step = 16 * p + kb
                                    lb, j = step // 4, step % 4
                                    if j == 0:
                                        pk1_box[0] = ps_pp.tile(
                                            [128, 512], F32, tag="pp",
                                            name=f"pk1_{lb}",
                                        )
                                    for kt in (2 * j, 2 * j + 1):
                                        kproj_mm(pk1_box[0], kh1, wkh1, 1, lb, kt)

                                emit_attn(p, vt_b, apB, kh_of_B, fillerB)
                                if p < 3:
                                    emit_vproj(vt_n, wvp_n, p + 1)
                                    vt_b, wvp_b = vt_n, wvp_n

                    # ---- pairs 4-7 (kh0 freed). Results + normalization
                    # inputs live in a manually-released pool that outlasts
                    # the nested scopes (read again by the output projection).
                    aprP = tc.alloc_tile_pool(name="aprP", bufs=1, side="right")
                    wvp4 = [
                        aprP.tile([128, 128], F32R, name=f"wv4e_{i}")
                        for i in range(KT)
                    ]
                    for i in range(KT):
                        nc.sync.dma_start(
                            wvp4[i][:], wv_d[128 * i : 128 * (i + 1), 512:640]
                        )
                    vt4 = aprP.tile([128, L], F32R, name="vt4e")
                    apr = [
                        aprP.tile([128, QS], F32R, name=f"apr{t}") for t in range(8)
                    ]
                    for t in range(4):
                        nc.sync.dma_start(
                            apr[t][:], apair_d[128 * t : 128 * (t + 1), :]
                        )
                    for t in range(4, 8):
                        qt[t] = aprP.tile([128, QS], F32R, name=f"qtB{t}")
                        nc.sync.dma_start(
                            qt[t][:], qsp_d[128 * (t - 4) : 128 * (t - 3), :]
                        )
                    rcpf = aprP.tile([NH, QS], F32R, name="rcpf")
                    nc.gpsimd.dma_start(rcpf[:], zrow_d[:])
                    nc.sync.dma_start(rcpf[0:8, :], rcp_d[0:8, :])
                    oh2 = aprP.tile([NH, 8 * 128], F32R, name="oh2t")
                    nc.gpsimd.dma_start(oh2[:], oh2_d[:])

                    def norm_tile(t):
                        pb = ps_pe.tile([128, QS], F32, tag="pe", name=f"pb{t}")
                        nc.tensor.matmul(
                            pb[:],
                            oh2[:, 128 * t : 128 * (t + 1)],
                            rcpf[:],
                            start=True,
                            stop=True,
                        )
                        nc.vector.tensor_mul(apr[t][:], apr[t][:], pb[:])

                    last_scs = [None]
                    with tc.tile_pool(name="attnC", bufs=1) as apC:
                        kh_of_C = lambda h: (kh1, h - 8)  # noqa: E731
                        vt_cur, wvp_cur = vt4, wvp4
                        emit_vproj(vt_cur, wvp_cur, 4)
                        for p in range(4, 8):
                            if p == 6:
                                for t in range(4):
                                    norm_tile(t)
                            if p == 7:
                                nc.sync.dma_start(rcpf[8:14, :], rcp_d[8:14, :])
                            if p < 7:
                                vt_nxt, wvp_nxt = make_vslot(p + 1, apC, "vtC", 2)

                                def fillerC(kb, p=p, vt_nxt=vt_nxt, wvp_nxt=wvp_nxt):
                                    nch, j = kb // 4, kb % 4
                                    for kt in (2 * j, 2 * j + 1):
                                        vproj_mm(vt_nxt, wvp_nxt, p + 1, nch, kt)
                                    if p == 7 - 1 and kb >= 13:
                                        pass

                            else:
                                fillerC = None
                            scs = emit_attn(
                                p, vt_cur, apC, kh_of_C, fillerC, direct=True
                            )
                            if p == 7:
                                last_scs[0] = scs
                            if p < 7:
                                vt_cur, wvp_cur = vt_nxt, wvp_nxt
                        for t in range(4, 7):
                            norm_tile(t)

            # ---- output proj, residual, LayerNorm
            with tc.tile_pool(name="fin", bufs=1) as fp:
                wot = [fp.tile([128, H], F32R, name=f"wot{t}") for t in range(8)]
                rest = [fp.tile([128, H], F32, name=f"rest{i}") for i in range(4)]
                epst = fp.tile([128, 1], F32, name="epst")
                nc.gpsimd.memset(epst[:], LN_EPS)
                for t in range(8):
                    nc.sync.dma_start(wot[t][:], wo_d[128 * t : 128 * (t + 1), :])
                for i in range(4):
                    nc.gpsimd.dma_start(rest[i][:], res_d[128 * i : 128 * (i + 1), :])
                # pair 7 normalization: K=1 broadcast straight from its sc rows
                for r in range(2):
                    sc = last_scs[0][r]
                    pb = ps_pe.tile([128, QS], F32, tag="pe", name=f"pbL{r}")
                    nc.tensor.matmul(
                        pb[0:64, :],
                        ones16[64:65, 0:64],
                        sc[64:65, :],
                        start=True,
                        stop=True,
                    )
                    nc.vector.tensor_mul(
                        apr[7][64 * r : 64 * (r + 1), :],
                        apr[7][64 * r : 64 * (r + 1), :],
                        pb[0:64, :],
                    )
                if has_gamma:
                    gam = fp.tile([128, H], F32, name="gam")
                    nc.sync.dma_start(gam[:], gam_d[:])
                if has_beta:
                    bet = fp.tile([128, H], F32, name="bet")
                    nc.sync.dma_start(bet[:], bet_d[:])
                for qcb in range(4):
                    x = fp.tile([128, H], F32, tag="x", bufs=2, name=f"x{qcb}")
                    for nch in range(2):
                        po = ps_pp.tile(
                            [128, 512], F32, tag="pp", name=f"po{qcb}_{nch}"
                        )
                        for t in range(8):
                            nc.tensor.matmul(
                                po[:],
                                apr[t][:, 128 * qcb : 128 * (qcb + 1)],
                                wot[t][:, 512 * nch : 512 * (nch + 1)],
                                start=(t == 0),
                                stop=(t == 7),
                            )
                        nc.vector.tensor_add(
                            x[:, 512 * nch : 512 * (nch + 1)],
                            po[:],
                            rest[qcb][:, 512 * nch : 512 * (nch + 1)],
                        )
                    # LayerNorm over the free axis (H)
                    mu = fp.tile([128, 1], F32, tag="mu", bufs=2, name=f"mu{qcb}")
                    nc.vector.reduce_sum(mu[:], x[:], axis=AX.X)
                    nc.vector.tensor_scalar_mul(mu[:], mu[:], 1.0 / H)
                    xm = fp.tile([128, H], F32, tag="xm", bufs=2, name=f"xm{qcb}")
                    nc.vector.tensor_scalar(xm[:], x[:], mu[:], None, op0=OP.subtract)
                    sq = fp.tile([128, H], F32, tag="sq", bufs=2, name=f"sq{qcb}")
                    var = fp.tile([128, 1], F32, tag="var", bufs=2, name=f"var{qcb}")
                    nc.scalar.activation(sq[:], xm[:], AF.Square, accum_out=var[:])
                    std = fp.tile([128, 1], F32, tag="std", bufs=2, name=f"std{qcb}")
                    nc.scalar.activation(
                        std[:], var[:], AF.Sqrt, scale=1.0 / H, bias=epst[:]
                    )
                    rstd = fp.tile([128, 1], F32, tag="rstd", bufs=2, name=f"rstd{qcb}")
                    nc.vector.reciprocal(rstd[:], std[:])
                    y = fp.tile([128, H], F32, tag="y", bufs=2, name=f"y{qcb}")
                    nc.vector.tensor_scalar(y[:], xm[:], rstd[:], None, op0=OP.mult)
                    if has_gamma:
                        nc.vector.tensor_mul(y[:], y[:], gam[:])
                    if has_beta:
                        nc.vector.tensor_add(y[:], y[:], bet[:])
                    nc.sync.dma_start(out_d[128 * qcb : 128 * (qcb + 1), :], y[:])
            aprP.release()

    nc.compile()
    return nc


def kernel(embeddings, mask, Wq, bq, Wk, bk, Wv, bv, Wo, bo, ln_gamma, ln_beta):
    from concourse.bass_utils import run_bass_kernel_spmd

    embeddings = np.asarray(embeddings, dtype=np.float32)
    mask = np.asarray(mask, dtype=np.float32)
    Wq, bq = np.asarray(Wq, np.float32), np.asarray(bq, np.float32)
    Wk, bk = np.asarray(Wk, np.float32), np.asarray(bk, np.float32)
    Wv, bv = np.asarray(Wv, np.float32), np.asarray(bv, np.float32)
    Wo, bo = np.asarray(Wo, np.float32), np.asarray(bo, np.float32)
    ln_gamma = np.asarray(ln_gamma, np.float32)
    ln_beta = np.asarray(ln_beta, np.float32)

    has_bias = bool(np.any(bq) or np.any(bk) or np.any(bv))
    has_mask = bool(np.any(mask))
    has_gamma = bool(np.any(ln_gamma != 1.0))
    has_beta = bool(np.any(ln_beta))

    key = (has_bias, has_mask, has_gamma, has_beta)
    if key not in _programs:
        _programs[key] = _build(*key)
    nc = _programs[key]

    cols = np.arange(8 * 128)
    heads_of_col = 2 * (cols // 128) + (cols % 128) // 64
    oh2 = (np.arange(NH)[:, None] == heads_of_col[None, :]).astype(np.float32)

    in_maps = []
    for c in range(NC):
        b, s = c // 4, c % 4
        e = embeddings[b]  # (L, H)
        embT = np.ascontiguousarray(e.T)  # (H, L)
        m = {
            "embT": embT,
            "embq": np.ascontiguousarray(embT[:, QS * s : QS * (s + 1)]),
            "wq": Wq,
            "wk": Wk,
            "wv": Wv,
            "wo": Wo,
            "onescol": np.ones((128, 64), dtype=np.float32),
            "zrow": np.zeros((NH, QS), dtype=np.float32),
            "oh2": oh2,
            "res": np.ascontiguousarray(e[QS * s : QS * (s + 1)] + bo[None, :]),
        }
        if has_mask:
            m["maskpk"] = np.ascontiguousarray(mask[b, 0, 0].reshape(LB, 128).T)
        if has_bias:
            m["bqr"] = bq[None, :].copy()
            m["bkr"] = bk[None, :].copy()
            m["bvr"] = bv[None, :].copy()
            m["onesrow"] = np.ones((1, L), dtype=np.float32)
        if has_gamma:
            m["gam"] = np.broadcast_to(ln_gamma, (128, H)).copy()
        if has_beta:
            m["bet"] = np.broadcast_to(ln_beta, (128, H)).copy()
        in_maps.append(m)

    r = run_bass_kernel_spmd(nc, in_maps, list(range(NC)))
    out = np.empty((B, L, H), dtype=np.float32)
    for c in range(NC):
        b, s = c // 4, c % 4
        out[b, QS * s : QS * (s + 1)] = r.results[c]["out"]
    return out

